# revision 9
# baseline (speedup 1.0000x reference)
"""DGGraphConv (GCN-style message passing) on 8 Trainium2 NeuronCores.

out = segment_sum(edge_weight * x[edge_src], edge_dst) @ W + bias

Reordering: aggregate raw x rows first, GEMM last.  No supp = x @ W
materialization and no collectives: every core receives the full x
(replicated, cast to fp16 host-side), gathers only the rows feeding its
destination-node shard with the custom SWDGE dma_gather, segment-sums them
via scatter-matmuls, and finishes with a small f32r GEMM.

v2 plan: dst nodes are assigned to (core, block, lane) bins host-side so
that every (block, src-chunk) cell holds at most 256 edges -> exactly 2
128-edge gather tiles per cell (~4% padding vs ~25% for fixed dst blocks).
Nodes whose edges overflow a cell are split across lanes; the host sums the
partial rows and removes the duplicated bias.  x is addressed in 4 equal
25000-row chunks (int16 gather indices).

Per-core device pipeline (identical SPMD program, data differs per core):
  for each super-block of SB_N blocks:
    - 4 dma_gather calls (one per src chunk) pull the super-block's edge
      source rows (fp16, 512B each) into SBUF (deep-buffered so the SDMA
      drain overlaps compute)
    for each block in the super-block:
      - per 128-edge tile: build S[e,n] = ew[e] * (lane[e]==n) with one
        fused DVE tensor_scalar (fp16), matmul-accumulate S.T @ G into PSUM
      - copy PSUM->SBUF, PE-transpose the [128,256] accumulator, then a
        3-matmul PSUM chain (bias broadcast + 2 f32r GEMMs with W), copy
        out, DMA the 128 output rows out
"""

import os

import numpy as np

import concourse.bass as bass
import concourse.mybir as mybir
import concourse.tile as tile
from concourse import bacc, bass_utils

N_NODES = 100000
N_EDGES = 800000
FEAT = 256
N_CORES = 8
P = 128
NCHUNK = 4
CHUNK = 25000                    # equal int16-addressable chunks
CELL_CAP = 256                   # max edges per (block, chunk) cell
CELL_TILES = CELL_CAP // P       # 2 tiles per cell
SB_N = 6                         # blocks per super-block

F32 = mybir.dt.float32
F32R = mybir.dt.float32r
F16 = mybir.dt.float16
I16 = mybir.dt.int16

GATHER_DT = F16
GATHER_NP = np.float16
EPI_DT = F32R   # final GEMM dtype

SINGLE_PACKET = os.environ.get("KERNEL_SP", "1") == "1"
GPOOL_BUFS = int(os.environ.get("KERNEL_GPB", "5"))
SPOOL_BUFS = int(os.environ.get("KERNEL_SPB", "16"))
BIAS_MM = os.environ.get("KERNEL_BIAS_MM", "1") == "1"

LAST_EXEC_TIME_NS = None


def _assign_nodes(d):
    """Sequential fill of nodes into 128-lane bins with per-chunk edge caps.

    d: [N_NODES, NCHUNK] edge counts per (dst node, src chunk).
    Returns placement arrays (node, bin, lane, take[4]).  A node whose
    edges do not fit the current bin is split across multiple lanes.
    """
    dl = d.tolist()
    p_node, p_bin, p_lane, p_take = [], [], [], []
    bin_id, lane = 0, 0
    fill = [0, 0, 0, 0]
    for n in range(N_NODES):
        rem = dl[n]
        first = True
        while True:
            if lane == P:
                bin_id += 1
                lane = 0
                fill = [0, 0, 0, 0]
            take = [min(rem[k], CELL_CAP - fill[k]) for k in range(NCHUNK)]
            if take[0] + take[1] + take[2] + take[3] > 0 or first:
                p_node.append(n)
                p_bin.append(bin_id)
                p_lane.append(lane)
                p_take.append(take)
                for k in range(NCHUNK):
                    fill[k] += take[k]
                    rem[k] -= take[k]
                lane += 1
                first = False
            if rem[0] == 0 and rem[1] == 0 and rem[2] == 0 and rem[3] == 0:
                break
            bin_id += 1
            lane = 0
            fill = [0, 0, 0, 0]
    return (np.asarray(p_node, dtype=np.int64),
            np.asarray(p_bin, dtype=np.int64),
            np.asarray(p_lane, dtype=np.int64),
            np.asarray(p_take, dtype=np.int64))


def _build_edge_plan(edge_src, edge_dst, edge_weight):
    """Balanced partition of edges into uniform 2-tile cells.

    Returns (NB, per_core, node_map):
      NB: blocks per core (uniform, program-defining).
      per_core[c] = (idx16 [P, 8*T] int16, win [P, T] f32, ew [P, T] f32)
      node_map: [N_CORES, NB*P] int64, node id per output lane or -1.
    Global tile order: for sb, for k, for b in sb, 2 tiles per cell.
    """
    src = edge_src.astype(np.int64)
    dst = edge_dst.astype(np.int64)
    ch = src // CHUNK
    d = np.zeros((N_NODES, NCHUNK), dtype=np.int64)
    np.add.at(d, (dst, ch), 1)

    p_node, p_bin, p_lane, p_take = _assign_nodes(d)
    B_tot = int(p_bin.max()) + 1
    NB = -(-B_tot // N_CORES)

    # distribute edges to placements: for each chunk, edges sorted by dst
    # align with placements repeated by take (placements are in node order)
    e_order = np.lexsort((ch, dst))
    ch_sorted = ch[e_order]
    e_bin = np.empty(N_EDGES, np.int64)
    e_lane = np.empty(N_EDGES, np.int64)
    np_rows = np.arange(len(p_node))
    for k in range(NCHUNK):
        rows = np.repeat(np_rows, p_take[:, k])
        idx_k = e_order[ch_sorted == k]
        assert len(rows) == len(idx_k)
        e_bin[idx_k] = p_bin[rows]
        e_lane[idx_k] = p_lane[rows]

    # global tile order: sb -> k -> block -> CELL_TILES
    n_sb = -(-NB // SB_N)
    tile_off = np.zeros((NB, NCHUNK), dtype=np.int64)
    tt = 0
    for sb in range(n_sb):
        for k in range(NCHUNK):
            for b in range(sb * SB_N, min((sb + 1) * SB_N, NB)):
                tile_off[b, k] = tt
                tt += CELL_TILES
    T = tt
    assert T == NB * NCHUNK * CELL_TILES

    core_of = e_bin // NB
    b_local_all = e_bin % NB
    per_core = []
    for c in range(N_CORES):
        sel = core_of == c
        src_c = src[sel]
        ew_c = edge_weight[sel]
        ch_c = ch[sel]
        bl = b_local_all[sel]
        ln = e_lane[sel]
        cell = bl * NCHUNK + ch_c
        order = np.lexsort((src_c, cell))       # src-sorted within cell
        cell_s = cell[order]
        cnt = np.bincount(cell_s, minlength=NB * NCHUNK)
        assert cnt.max() <= CELL_CAP, cnt.max()
        starts = np.concatenate([[0], np.cumsum(cnt)[:-1]])
        rank = np.arange(len(cell_s)) - starts[cell_s]
        pos = tile_off.reshape(-1)[cell_s] * P + rank

        srcl_pad = np.zeros(T * P, dtype=np.int16)
        win_pad = np.zeros(T * P, dtype=np.float32)
        ew_pad = np.zeros(T * P, dtype=np.float32)
        srcl_pad[pos] = (src_c[order] - ch_c[order] * CHUNK).astype(np.int16)
        win_pad[pos] = ln[order]
        ew_pad[pos] = ew_c[order]

        # idx16 layout: tile t's 128 idxs occupy columns [8t, 8t+8);
        # idx j -> [16r + (j%16), 8t + j//16] replicated for r in 0..7
        blk16 = srcl_pad.reshape(T, 8, 16)
        idx16 = np.zeros((P, 8 * T), dtype=np.int16)
        lanes = blk16.transpose(2, 0, 1).reshape(16, 8 * T)
        for r in range(8):
            idx16[16 * r:16 * (r + 1)] = lanes
        per_core.append((
            np.ascontiguousarray(idx16),
            np.ascontiguousarray(win_pad.reshape(T, P).T),
            np.ascontiguousarray(ew_pad.reshape(T, P).T),
        ))

    node_map = np.full((N_CORES, NB * P), -1, dtype=np.int64)
    pc = p_bin // NB
    ppos = (p_bin % NB) * P + p_lane
    node_map[pc, ppos] = p_node
    return NB, per_core, node_map


def _build_nc(nb):
    T = nb * NCHUNK * CELL_TILES
    n_sb = -(-nb // SB_N)
    nc = bacc.Bacc("TRN2", target_bir_lowering=False, debug=False,
                   num_swdge_queues=4)

    x16 = nc.dram_tensor("x16", [N_NODES, FEAT], GATHER_DT,
                         kind="ExternalInput").ap()
    w = nc.dram_tensor("w", [2 * P, FEAT], EPI_DT, kind="ExternalInput").ap()
    if BIAS_MM:
        bias_in = nc.dram_tensor("bias_in", [1, FEAT], EPI_DT,
                                 kind="ExternalInput").ap()
        ones_in = nc.dram_tensor("ones_in", [1, P], EPI_DT,
                                 kind="ExternalInput").ap()
    else:
        bias_in = nc.dram_tensor("bias_in", [P, FEAT], F32,
                                 kind="ExternalInput").ap()
    iota = nc.dram_tensor("iota", [P, P], GATHER_DT, kind="ExternalInput").ap()
    ident = nc.dram_tensor("ident", [P, P], F32, kind="ExternalInput").ap()
    idx16 = nc.dram_tensor("idx16", [P, 8 * T], I16, kind="ExternalInput").ap()
    dst_win = nc.dram_tensor("dst_win", [P, T], F32, kind="ExternalInput").ap()
    ew_in = nc.dram_tensor("ew", [P, T], F32, kind="ExternalInput").ap()
    out = nc.dram_tensor("out", [nb * P, FEAT], F32, kind="ExternalOutput").ap()

    gmax = CELL_TILES * SB_N     # tiles per gather call (full super-block)

    with tile.TileContext(nc) as tc:
        with (
            tc.tile_pool(name="consts", bufs=1) as cpool,
            tc.tile_pool(name="gpool", bufs=GPOOL_BUFS) as gpool,
            tc.tile_pool(name="spool", bufs=SPOOL_BUFS) as spool,
            tc.tile_pool(name="accsb", bufs=2) as accsb_pool,
            tc.tile_pool(name="outsb", bufs=3) as outsb_pool,
            tc.tile_pool(name="psacc", bufs=2, space="PSUM") as ps_acc,
            tc.tile_pool(name="pstp", bufs=2, space="PSUM") as ps_tp,
            tc.tile_pool(name="psout", bufs=2, space="PSUM") as ps_out,
        ):
            w_sb = cpool.tile([P, 2 * FEAT], EPI_DT)
            nc.sync.dma_start(out=w_sb[:, 0:FEAT], in_=w[0:P, :])
            nc.sync.dma_start(out=w_sb[:, FEAT:2 * FEAT], in_=w[P:2 * P, :])
            iota_sb = cpool.tile([P, P], GATHER_DT)
            nc.sync.dma_start(out=iota_sb[:], in_=iota[:])
            ident_sb = cpool.tile([P, P], F32)
            nc.sync.dma_start(out=ident_sb[:], in_=ident[:])
            idx_sb = cpool.tile([P, 8 * T], I16)
            nc.sync.dma_start(out=idx_sb[:], in_=idx16[:])
            dst_sb = cpool.tile([P, T], F32)
            nc.sync.dma_start(out=dst_sb[:], in_=dst_win[:])
            ew_sb = cpool.tile([P, T], F32)
            nc.sync.dma_start(out=ew_sb[:], in_=ew_in[:])
            if BIAS_MM:
                bias_sb = cpool.tile([1, FEAT], EPI_DT)
                nc.sync.dma_start(out=bias_sb[:], in_=bias_in[:])
                ones_sb = cpool.tile([1, P], EPI_DT)
                nc.sync.dma_start(out=ones_sb[:], in_=ones_in[:])
            else:
                bias_sb = cpool.tile([P, FEAT], F32)
                nc.sync.dma_start(out=bias_sb[:], in_=bias_in[:])

            tt = 0          # global tile counter (gather order)
            for sb in range(n_sb):
                blocks = list(range(sb * SB_N, min((sb + 1) * SB_N, nb)))
                nblk = len(blocks)
                # gather: one call per chunk
                g_k = [None] * NCHUNK
                base_k = [0] * NCHUNK
                for k in range(NCHUNK):
                    n = CELL_TILES * nblk
                    base_k[k] = tt
                    g = gpool.tile([P, n * FEAT], GATHER_DT,
                                   tag=f"g{k}", padded_shape=[P, gmax * FEAT],
                                   name=f"g{k}")
                    g_k[k] = g
                    g3 = g[:].rearrange("p (c f) -> p c f", f=FEAT)
                    nc.gpsimd.dma_gather(
                        out_ap=g3,
                        in_ap=x16[k * CHUNK:min((k + 1) * CHUNK, N_NODES), :],
                        idxs_ap=idx_sb[:, 8 * tt:8 * (tt + n)],
                        num_idxs=n * P,
                        num_idxs_reg=n * P,
                        elem_size=FEAT,
                        single_packet=SINGLE_PACKET,
                        queue_num=k,
                    )
                    tt += n

                # compute per block
                for bpos, b in enumerate(blocks):
                    acc = ps_acc.tile([P, FEAT], F32, tag="acc")
                    done = 0
                    ntb = NCHUNK * CELL_TILES
                    for k in range(NCHUNK):
                        gcol = base_k[k] + bpos * CELL_TILES
                        for t in range(CELL_TILES):
                            s = spool.tile([P, P], GATHER_DT, tag="s")
                            nc.vector.tensor_scalar(
                                out=s[:],
                                in0=iota_sb[:],
                                scalar1=dst_sb[:, gcol + t:gcol + t + 1],
                                scalar2=ew_sb[:, gcol + t:gcol + t + 1],
                                op0=mybir.AluOpType.is_equal,
                                op1=mybir.AluOpType.mult,
                            )
                            goff = (bpos * CELL_TILES + t) * FEAT
                            nc.tensor.matmul(
                                out=acc[:],
                                lhsT=s[:],
                                rhs=g_k[k][:, goff:goff + FEAT],
                                start=(done == 0),
                                stop=(done == ntb - 1),
                            )
                            done += 1

                    acc_sb = accsb_pool.tile([P, FEAT], F32, tag="acc_sb")
                    nc.scalar.copy(out=acc_sb[:], in_=acc[:])
                    accT_sb = accsb_pool.tile([P, FEAT], EPI_DT, tag="accT_sb")
                    for h in range(2):
                        tp = ps_tp.tile([P, P], F32, tag="tp")
                        nc.tensor.transpose(
                            out=tp[:], in_=acc_sb[:, h * P:(h + 1) * P],
                            identity=ident_sb[:])
                        nc.scalar.copy(
                            out=accT_sb[:, h * P:(h + 1) * P], in_=tp[:])
                    outp = ps_out.tile([P, FEAT], F32, tag="outp")
                    if BIAS_MM:
                        nc.tensor.matmul(
                            out=outp[:], lhsT=ones_sb[:], rhs=bias_sb[:],
                            start=True, stop=False)
                    for h in range(2):
                        nc.tensor.matmul(
                            out=outp[:],
                            lhsT=accT_sb[:, h * P:(h + 1) * P],
                            rhs=w_sb[:, h * FEAT:(h + 1) * FEAT],
                            start=(h == 0 and not BIAS_MM),
                            stop=(h == 1),
                        )
                    out_t = outsb_pool.tile([P, FEAT], F32, tag="out_t")
                    if BIAS_MM:
                        nc.scalar.copy(out=out_t[:], in_=outp[:])
                    else:
                        nc.vector.tensor_tensor(
                            out=out_t[:], in0=outp[:], in1=bias_sb[:],
                            op=mybir.AluOpType.add)
                    nc.sync.dma_start(out=out[b * P:(b + 1) * P, :], in_=out_t[:])
            assert tt == T

    nc.compile()
    return nc


def _install_ntff_hook():
    """Register the axon NTFF profile hook that this image's antenv lacks."""
    import sys
    import types

    try:
        from antenv.axon_hooks import get_axon_ntff_profile_hook  # noqa: F401
        return True
    except ImportError:
        pass
    try:
        import antenv
        from trn_agent_boot.trn_boot import _ntff_profile_via_ctypes
    except ImportError:
        return False
    hook = _ntff_profile_via_ctypes("/opt/axon/libaxon_pjrt.so")
    if hook is None:
        return False
    mod = types.ModuleType("antenv.axon_hooks")
    mod._hook = hook
    mod.set_axon_ntff_profile_hook = lambda h: setattr(mod, "_hook", h)
    mod.get_axon_ntff_profile_hook = lambda: mod._hook
    sys.modules["antenv.axon_hooks"] = mod
    antenv.axon_hooks = mod
    return True


_NC_CACHE = {}


def _get_nc(nb):
    if nb not in _NC_CACHE:
        _NC_CACHE[nb] = _build_nc(nb)
    return _NC_CACHE[nb]


def kernel(x, weight, bias, edge_weight, edge_src, edge_dst):
    global LAST_EXEC_TIME_NS
    x = np.ascontiguousarray(np.asarray(x, dtype=np.float32))
    weight = np.ascontiguousarray(np.asarray(weight, dtype=np.float32))
    bias = np.asarray(bias, dtype=np.float32)
    edge_weight = np.asarray(edge_weight, dtype=np.float32)
    edge_src = np.asarray(edge_src, dtype=np.int32)
    edge_dst = np.asarray(edge_dst, dtype=np.int32)

    nb, per_core, node_map = _build_edge_plan(edge_src, edge_dst, edge_weight)
    nc = _get_nc(nb)

    x_g = x.astype(GATHER_NP)
    if BIAS_MM:
        bias_arr = np.ascontiguousarray(bias.reshape(1, FEAT))
    else:
        bias_arr = np.ascontiguousarray(
            np.broadcast_to(bias.reshape(1, FEAT), (P, FEAT)))
    iota = np.ascontiguousarray(np.broadcast_to(
        np.arange(P, dtype=np.float32).reshape(1, P), (P, P))).astype(GATHER_NP)
    ident = np.eye(P, dtype=np.float32)

    in_maps = []
    for c in range(N_CORES):
        idx16_c, win_c, ew_c = per_core[c]
        m_ = {
            "x16": x_g,
            "w": weight,
            "bias_in": bias_arr,
            "iota": iota,
            "ident": ident,
            "idx16": idx16_c,
            "dst_win": win_c,
            "ew": ew_c,
        }
        if BIAS_MM:
            m_["ones_in"] = np.ones((1, P), dtype=np.float32)
        in_maps.append(m_)

    trace = os.environ.get("KERNEL_TRACE", "0") == "1"
    kw = {}
    if trace:
        if _install_ntff_hook():
            bass_utils.upload_artifacts = lambda tmpdir: tmpdir
            kw = dict(trace=True, trace_cores=list(range(N_CORES)))
        else:
            print("KERNEL_TRACE requested but NTFF hook unavailable")
    res = bass_utils.run_bass_kernel_spmd(
        nc, in_maps, core_ids=list(range(N_CORES)), **kw)
    LAST_EXEC_TIME_NS = res.exec_time_ns
    if trace:
        print(f"KERNEL_EXEC_TIME_NS: {res.exec_time_ns}")
        print(f"KERNEL_MEAN_EXEC_TIME_NS: {res.mean_exec_time_ns}")
        if res.instructions_and_trace is not None:
            print(f"KERNEL_TRACE_PATH: {res.instructions_and_trace[1]}")

    # host reassembly: sum partial rows per node, remove duplicated bias
    all_nodes = []
    all_rows = []
    for c in range(N_CORES):
        nm = node_map[c]
        val = nm >= 0
        rows = np.asarray(res.results[c]["out"])[:nb * P]
        all_nodes.append(nm[val])
        all_rows.append(rows[val])
    all_nodes = np.concatenate(all_nodes)
    all_rows = np.concatenate(all_rows, axis=0)
    o = np.argsort(all_nodes, kind="stable")
    nodes_s = all_nodes[o]
    bounds = np.flatnonzero(np.r_[True, nodes_s[1:] != nodes_s[:-1]])
    assert len(bounds) == N_NODES
    sums = np.add.reduceat(all_rows[o], bounds, axis=0)
    counts = np.diff(np.r_[bounds, len(nodes_s)])
    out = sums - (counts - 1)[:, None].astype(np.float32) * bias.reshape(1, FEAT)
    return np.ascontiguousarray(out.astype(np.float32))
